# revision 2
# baseline (speedup 1.0000x reference)
"""Trainium2 Bass kernel for nn_MultiHeadEDT — v3.

Pure data parallel over batch B=131072 across 8 NeuronCores (16384
rows/core). Major changes vs v2 (772µs baseline):

- W2 fold: attn already carries the LN1 rstd scaling (folded into the
  softmax normalizer), so y = attn @ W2 with W2 = povc_bd @ fWg
  (16 x 1024, exact host-side fold). This removes the zT matmuls and
  replaces the contraction-512 final projection (32 MMs/block) with 8
  K=16 MMs/block.
- No on-device x transposes (was ~35% of PE time): host ships x.T in
  fp8e4m3 (xt8, 16MB/core) laid out for DoubleRow matmuls.
- q-projection in fp8 DoubleRow perf mode (2 contraction chunks per
  instruction, ~1.4-1.5x PE throughput). The 512 q columns only feed
  ||q||^2 so fp8 noise averages out; raw logits in fp8 measured at
  rel_l2 3.6e-3 end-to-end on host simulation (gate 2e-2).
- Weights scaled by 64 into fp8 range (qW sigma=0.02 would be
  subnormal); raw = rawu * rnorm is scale-invariant, qb cross terms
  host-rescaled (x64 for linear, x4096 for quadratic).
- xlo eliminated (residual uses bf16 x only, +0.3% L2), y written bf16
  (+0.2% L2), halving in+out DMA: 16+32+32 = 80MB/core vs 128MB.

Host-side algebraic folds (exact, fp32):
  knS[h]  = (pk[h]/||pk[h]||) * clip(scale,1,50)
  qWk[h]  = qW[h] @ knS[h].T
  qWq[h]  = qW[h] @ qb[h]          (cross term so ||q|| includes qb)
  povW2[h]= pv[h] @ oW[h] + ob[h]; povC = povW2 - rowmean(povW2)
  Gc[h]   = povC[h] povC[h]^T / A
  W2      = povc_bd @ (lng_flat[:,None] * fW); fb2 = fb + lnb_flat @ fW
"""

import numpy as np
import ml_dtypes

B, D, H, A, P, T = 131072, 1024, 4, 128, 4, 32
TAU_MIN, TAU_MAX = 0.1, 5.0
EPS = 1e-5
NCORES = 8
BLOC = B // NCORES
NSUB = 4
RBLK = 128 * NSUB
NBLK = BLOC // RBLK
KD = D // 128                 # 8 contraction chunks for q-proj
KD2 = KD // 2                 # 4 DoubleRow chunk-pairs
SC = 64.0                     # fp8 weight scale (qW sigma=0.02 -> x64)
LN2_F32 = float(np.log(2.0))
# ln(m)/m deg-5 fit on [1,2]; nested form g=(g+c)*m, highest power first
LN_C = [0.2051921279531045, -1.8069928487438482, 6.502359993057587,
        -12.111644716066102, 11.908857088542383, -4.697566486562566]
MAGIC_P1 = 0x5f3759e0         # quake magic + 1 (for xor/add negation)

_cache = {}


def _bf(a):
    return np.ascontiguousarray(np.asarray(a, np.float32)).astype(ml_dtypes.bfloat16)


def _f8(a):
    return np.ascontiguousarray(np.asarray(a, np.float32)).astype(ml_dtypes.float8_e4m3)


def _build(flags, nblk=NBLK, tune=None):
    """flags = (qb_nz, tb1_nz, tb2_nz, fln_nz, fb2_nz)."""
    import concourse.bass as bass
    import concourse.mybir as mybir
    import concourse.tile as tile
    from concourse.bacc import Bacc

    qb_nz, tb1_nz, tb2_nz, fln_nz, fb2_nz = flags
    RW = H * P + (H if qb_nz else 0)   # raw cols consumed by chain
    EW = 32 if qb_nz else 16           # ext matmul out width (DoubleRow pad)
    tu = dict(pxb=6, pxb2=3, pyf=3, pyo=3, psm=3, psq=2,
              ppt=2, ppbig=2, ppy=3,
              gb=2, delay=2, ssq_acc=4, yss_acc=2)
    if tune:
        tu.update(tune)
    f32 = mybir.dt.float32
    bf16 = mybir.dt.bfloat16
    f8e4 = mybir.dt.float8e4
    i32 = mybir.dt.int32
    Act = mybir.ActivationFunctionType
    Op = mybir.AluOpType
    DR = mybir.MatmulPerfMode.DoubleRow

    nc = Bacc("TRN2", debug=False, enable_asserts=False,
              target_bir_lowering=False, num_devices=NCORES)

    # ---- DRAM I/O ----
    xt8_d = nc.dram_tensor("xt8", (128, KD2, 2, BLOC), f8e4, kind="ExternalInput").ap()
    xhi_d = nc.dram_tensor("xhi", (BLOC, D), bf16, kind="ExternalInput").ap()
    y_d = nc.dram_tensor("y", (BLOC, D), bf16, kind="ExternalOutput").ap()
    qw8_d = nc.dram_tensor("qw8", (128, KD2, 2, 512), f8e4, kind="ExternalInput").ap()
    ext8_d = nc.dram_tensor("ext8", (128, KD2, 2, EW), f8e4, kind="ExternalInput").ap()
    w2_d = nc.dram_tensor("w2", (H * P, D), bf16, kind="ExternalInput").ap()
    ident_d = nc.dram_tensor("ident", (128, 128), bf16, kind="ExternalInput").ap()
    tw1_d = nc.dram_tensor("tw1r", (128, H * T), f32, kind="ExternalInput").ap()
    tw2_d = nc.dram_tensor("tw2r", (128, H * T), f32, kind="ExternalInput").ap()
    gcb_d = nc.dram_tensor("gcb", (128, P, H, P), f32, kind="ExternalInput").ap()
    opt_d = {}
    if qb_nz:
        opt_d["qbkr"] = nc.dram_tensor("qbkr", (128, H * P), f32, kind="ExternalInput").ap()
        opt_d["qbn2r"] = nc.dram_tensor("qbn2r", (128, H), f32, kind="ExternalInput").ap()
    if tb1_nz:
        opt_d["tb1r"] = nc.dram_tensor("tb1r", (128, H * T), f32, kind="ExternalInput").ap()
    if tb2_nz:
        opt_d["tb2r"] = nc.dram_tensor("tb2r", (128, H), f32, kind="ExternalInput").ap()
    if fln_nz:
        opt_d["flngr"] = nc.dram_tensor("flngr", (128, D), f32, kind="ExternalInput").ap()
        opt_d["flnbr"] = nc.dram_tensor("flnbr", (128, D), f32, kind="ExternalInput").ap()
    if fb2_nz:
        opt_d["fb2r"] = nc.dram_tensor("fb2r", (128, D), f32, kind="ExternalInput").ap()

    xtv = xt8_d  # [128, KD2, 2, BLOC]: rows are last dim
    xhv = xhi_d.rearrange("(n s p) d -> n p s d", s=NSUB, p=128)
    yv = y_d.rearrange("(n s p) d -> n p s d", s=NSUB, p=128)

    from contextlib import ExitStack
    with tile.TileContext(nc) as tc, ExitStack() as stack:
        cpool = stack.enter_context(tc.tile_pool(name="consts", bufs=1))
        pxb = stack.enter_context(tc.tile_pool(name="pxb", bufs=tu["pxb"]))
        pxb2 = stack.enter_context(tc.tile_pool(name="pxb2", bufs=tu["pxb2"]))
        pyf = stack.enter_context(tc.tile_pool(name="pyf", bufs=tu["pyf"]))
        pyo = stack.enter_context(tc.tile_pool(name="pyo", bufs=tu["pyo"]))
        psm = stack.enter_context(tc.tile_pool(name="psm", bufs=tu["psm"]))
        psq = stack.enter_context(tc.tile_pool(name="psq", bufs=tu["psq"]))
        pp_t = stack.enter_context(tc.tile_pool(name="pp_t", bufs=tu["ppt"], space="PSUM"))
        pp_q = stack.enter_context(tc.tile_pool(name="pp_q", bufs=tu["ppbig"], space="PSUM"))
        pp_y = stack.enter_context(tc.tile_pool(name="pp_y", bufs=tu["ppy"], space="PSUM"))

        # ---- load constants once ----
        qw8 = cpool.tile([128, KD2, 2, 512], f8e4)
        nc.sync.dma_start(qw8[:], qw8_d[:])
        ext8 = cpool.tile([128, KD2, 2, EW], f8e4)
        nc.sync.dma_start(ext8[:], ext8_d[:])
        w2sb = cpool.tile([H * P, D], bf16)
        nc.sync.dma_start(w2sb[:], w2_d[:])
        ident = cpool.tile([128, 128], bf16)
        nc.sync.dma_start(ident[:], ident_d[:])
        tw1r = cpool.tile([128, H * T], f32)
        nc.sync.dma_start(tw1r[:], tw1_d[:])
        tw2r = cpool.tile([128, H * T], f32)
        nc.sync.dma_start(tw2r[:], tw2_d[:])
        gcb = cpool.tile([128, P, H, P], f32)
        nc.sync.dma_start(gcb[:], gcb_d[:])
        opt = {}
        for k, dap in opt_d.items():
            t = cpool.tile(list(dap.shape), f32, name=k + "_sb")
            nc.sync.dma_start(t[:], dap[:])
            opt[k] = t

        def quake(dst, src, shape, newton=None):
            newton = tu.get("newton", 1) if newton is None else newton
            """dst = 1/sqrt(src), fp32 DVE-only (bit-trick + Newton)."""
            sh = psm.tile(shape, i32, tag="qk_sh")
            nc.vector.tensor_scalar(sh[:], src.bitcast(i32), 1, -1,
                                    Op.logical_shift_right, Op.bitwise_xor)
            y = psm.tile(shape, f32, tag="qk_y")
            nc.vector.tensor_scalar_add(y.bitcast(i32)[:], sh[:], MAGIC_P1)
            vh = psm.tile(shape, f32, tag="qk_vh")
            nc.vector.tensor_scalar_mul(vh[:], src, 0.5)
            for it in range(newton):
                t1 = psm.tile(shape, f32, tag="qk_t")
                nc.vector.tensor_tensor(t1[:], y[:], y[:], Op.mult)
                nc.vector.tensor_tensor(t1[:], t1[:], vh[:], Op.mult)
                nc.vector.tensor_scalar(t1[:], t1[:], -1.0, 1.5, Op.mult, Op.add)
                yn = dst if it == newton - 1 else psm.tile(shape, f32, tag="qk_y")
                nc.vector.tensor_tensor(yn[:], y[:], t1[:], Op.mult)
                y = yn

        def phase_a(blk, ssq_dst, raw_dst):
            # ---- load xT fp8 block (rows = last dim; split for overlap) ----
            xt = pxb.tile([128, KD2, 2, RBLK], f8e4, name="xt")
            nc.sync.dma_start(xt[:, :, :, 0:RBLK // 2],
                              xtv[:, :, :, blk * RBLK:blk * RBLK + RBLK // 2])
            nc.sync.dma_start(xt[:, :, :, RBLK // 2:RBLK],
                              xtv[:, :, :, blk * RBLK + RBLK // 2:(blk + 1) * RBLK])

            # ---- q projection + raw logits via fp8 DoubleRow ----
            ext_ps = pp_t.tile([128, NSUB, EW], f32, tag="ext", name="ext_ps")
            for s in range(NSUB):
                q_ps = pp_q.tile([128, 512], f32, tag="q", name="q_ps")
                for dcp in range(KD2):
                    lhs = xt[:, dcp, :, s * 128:(s + 1) * 128]
                    nc.tensor.matmul(q_ps[:], lhs, qw8[:, dcp],
                                     start=(dcp == 0), stop=(dcp == KD2 - 1),
                                     perf_mode=DR)
                    nc.tensor.matmul(ext_ps[:, s, :], lhs, ext8[:, dcp],
                                     start=(dcp == 0), stop=(dcp == KD2 - 1),
                                     perf_mode=DR)
                if s < tu.get("ssq_acc", 2):
                    for h in range(H):
                        sqs = psq.tile([128, A], bf16, tag="sqs", name="sqs")
                        nc.scalar.activation(sqs[:], q_ps[:, h * A:(h + 1) * A],
                                             Act.Square,
                                             accum_out=ssq_dst[:, s, h:h + 1])
                else:
                    sqs = psq.tile([128, 512], bf16, tag="sqs2", name="sqs2")
                    nc.scalar.activation(sqs[:], q_ps[:], Act.Square)
                    nc.vector.tensor_reduce(
                        ssq_dst[:, s, :],
                        sqs.rearrange("p (h a) -> p h a", h=H)[:],
                        axis=mybir.AxisListType.X, op=Op.add)
            nc.vector.tensor_copy(raw_dst[:], ext_ps[:, :, 0:RW])
            return dict(blk=blk)

        def chain(grp):
            S = grp["S"]
            ssq, raw = grp["ssq"], grp["raw"]
            # ---- 1/||q|| (incl. qb cross term when qb!=0) ----
            # device q values are 64x true; ssq is 4096x; raw invariant.
            if qb_nz:
                ssqe = psm.tile([128, S, H], f32, name="ssqe")
                nc.vector.scalar_tensor_tensor(ssqe[:], raw[:, :, H * P:], 2.0 * SC,
                                               ssq[:], Op.mult, Op.add)
                nc.vector.tensor_tensor(
                    ssqe[:], ssqe[:],
                    opt["qbn2r"].unsqueeze(1).broadcast_to([128, S, H]), Op.add)
                ssq = ssqe
            rnorm = psm.tile([128, S, H], f32, name="rnorm")
            quake(rnorm[:], ssq[:], [128, S, H])

            # ---- raw = (rawU + qbk) * rnorm ----
            rawv = raw[:, :, 0:H * P].rearrange("p s (h q) -> p s h q", h=H)
            raw_sb = rawv  # scaled in place
            if qb_nz:
                nc.vector.tensor_tensor(
                    raw_sb, rawv,
                    opt["qbkr"].rearrange("p (h q) -> p h q", h=H)
                    .unsqueeze(1).broadcast_to([128, S, H, P]), Op.add)
                nc.vector.tensor_tensor(
                    raw_sb, raw_sb,
                    rnorm.unsqueeze(3).broadcast_to([128, S, H, P]), Op.mult)
            else:
                nc.vector.tensor_tensor(
                    raw_sb, rawv,
                    rnorm.unsqueeze(3).broadcast_to([128, S, H, P]), Op.mult)

            # ---- softmax-1 stats + entropy; |raw|<=50 so exp cannot overflow
            # fp32 and the max-subtraction is skipped (H = ln(se) - dote/se
            # is the shift-invariant identity evaluated unshifted) ----
            ee = psm.tile([128, S, H, P], f32, name="ee")
            nc.scalar.activation(ee[:], raw_sb, Act.Exp)
            se = psm.tile([128, S, H], f32, name="se")
            nc.vector.tensor_reduce(se[:], ee[:], axis=mybir.AxisListType.X, op=Op.add)
            nc.vector.tensor_tensor(ee[:], ee[:], raw_sb, Op.mult)
            dote = psm.tile([128, S, H], f32, name="dote")
            nc.vector.tensor_reduce(dote[:], ee[:], axis=mybir.AxisListType.X, op=Op.add)
            rse = psm.tile([128, S, H], f32, name="rse")
            nc.vector.reciprocal_approx_fast(rse[:], se[:])
            # lnse = ln(se): exponent + mantissa-poly (any positive se)
            efv = psm.tile([128, S, H], i32, name="efv")
            nc.vector.tensor_scalar(efv[:], se.bitcast(i32)[:], 23, 0x4B000000,
                                    Op.logical_shift_right, Op.bitwise_or)
            ef = psm.tile([128, S, H], f32, name="ef")
            nc.vector.tensor_scalar_add(ef[:], efv.bitcast(f32)[:], -8388735.0)
            mant = psm.tile([128, S, H], f32, name="mant")
            nc.vector.tensor_scalar(mant.bitcast(i32)[:], se.bitcast(i32)[:],
                                    0x007FFFFF, 0x3F800000,
                                    Op.bitwise_and, Op.bitwise_or)
            lg = psm.tile([128, S, H], f32, name="lg")
            nc.vector.tensor_scalar_mul(lg[:], mant[:], LN_C[0])
            for cj in LN_C[1:]:
                nc.vector.scalar_tensor_tensor(lg[:], lg[:], cj, mant[:],
                                               Op.add, Op.mult)
            lnse = psm.tile([128, S, H], f32, name="lnse")
            nc.vector.scalar_tensor_tensor(lnse[:], ef[:], LN2_F32, lg[:],
                                           Op.mult, Op.add)
            tq = psm.tile([128, S, H], f32, name="tq")
            nc.vector.tensor_tensor(tq[:], dote[:], rse[:], Op.mult)
            ent = psm.tile([128, S, H], f32, name="ent")
            nc.vector.tensor_tensor(ent[:], lnse[:], tq[:], Op.subtract)

            # ---- tiny MLP -> 1/tau (1/lnP folded into tw1r) ----
            hm = psm.tile([128, S, H, T], bf16, name="hm")
            nc.vector.tensor_tensor(
                hm[:], ent.unsqueeze(3).broadcast_to([128, S, H, T]),
                tw1r.rearrange("p (h t) -> p h t", h=H)
                .unsqueeze(1).broadcast_to([128, S, H, T]), Op.mult)
            if tb1_nz:
                nc.vector.tensor_tensor(
                    hm[:], hm[:],
                    opt["tb1r"].rearrange("p (h t) -> p h t", h=H)
                    .unsqueeze(1).broadcast_to([128, S, H, T]), Op.add)
            nc.vector.tensor_scalar_max(hm[:], hm[:], 0.0)
            nc.vector.tensor_tensor(
                hm[:], hm[:],
                tw2r.rearrange("p (h t) -> p h t", h=H)
                .unsqueeze(1).broadcast_to([128, S, H, T]), Op.mult)
            u = psm.tile([128, S, H], f32, name="u")
            nc.vector.tensor_reduce(u[:], hm[:], axis=mybir.AxisListType.X, op=Op.add)
            if tb2_nz:
                nc.vector.tensor_tensor(
                    u[:], u[:],
                    opt["tb2r"].unsqueeze(1).broadcast_to([128, S, H]), Op.add)
            en = psm.tile([128, S, H], f32, name="en")
            nc.scalar.activation(en[:], u[:], Act.Exp, scale=-1.0)
            numv = psm.tile([128, S, H], f32, name="numv")
            nc.vector.tensor_scalar_add(numv[:], en[:], 1.0)
            denv = psm.tile([128, S, H], f32, name="denv")
            nc.vector.tensor_scalar(denv[:], en[:], TAU_MIN, TAU_MAX, Op.mult, Op.add)
            rden = psm.tile([128, S, H], f32, name="rden")
            nc.vector.reciprocal_approx_fast(rden[:], denv[:])
            itau = psm.tile([128, S, H], f32, name="itau")
            nc.vector.tensor_tensor(itau[:], numv[:], rden[:], Op.mult)

            # ---- softmax-2 numerators (|zz| can reach 100+: keep max-sub) ----
            zz = psm.tile([128, S, H, P], f32, name="zz")
            nc.vector.tensor_tensor(zz[:], raw_sb,
                                    itau.unsqueeze(3).broadcast_to([128, S, H, P]),
                                    Op.mult)
            m2 = psm.tile([128, S, H], f32, name="m2")
            nc.vector.tensor_reduce(m2[:], zz[:], axis=mybir.AxisListType.X, op=Op.max)
            nc.vector.tensor_tensor(zz[:], zz[:],
                                    m2.unsqueeze(3).broadcast_to([128, S, H, P]),
                                    Op.subtract)
            e2 = psm.tile([128, S, H, P], f32, name="e2")
            nc.scalar.activation(e2[:], zz[:], Act.Exp)
            se2 = psm.tile([128, S, H], f32, name="se2")
            nc.vector.tensor_reduce(se2[:], e2[:], axis=mybir.AxisListType.X, op=Op.add)
            rse2 = psm.tile([128, S, H], f32, name="rse2")
            nc.vector.reciprocal_approx_fast(rse2[:], se2[:])

            # ---- LN1 var via quadratic form: w = e2 Gc e2^T ----
            eg = psm.tile([128, S, H, P], f32, name="eg")
            nc.vector.tensor_tensor(
                eg[:], e2[:, :, :, 0:1].broadcast_to([128, S, H, P]),
                gcb[:, 0].unsqueeze(1).broadcast_to([128, S, H, P]), Op.mult)
            for p in range(1, P):
                tp = psm.tile([128, S, H, P], f32, tag="eg_t", name="eg_t")
                nc.vector.tensor_tensor(
                    tp[:], e2[:, :, :, p:p + 1].broadcast_to([128, S, H, P]),
                    gcb[:, p].unsqueeze(1).broadcast_to([128, S, H, P]), Op.mult)
                nc.vector.tensor_tensor(eg[:], eg[:], tp[:], Op.add)
            ed2 = psm.tile([128, S, H, P], f32, name="ed2")
            nc.vector.tensor_tensor(ed2[:], eg[:], e2[:], Op.mult)
            w = psm.tile([128, S, H], f32, name="w")
            nc.vector.tensor_reduce(w[:], ed2[:], axis=mybir.AxisListType.X, op=Op.add)
            rse2sq = psm.tile([128, S, H], f32, name="rse2sq")
            nc.vector.tensor_tensor(rse2sq[:], rse2[:], rse2[:], Op.mult)
            varv = psm.tile([128, S, H], f32, name="varv")
            nc.vector.tensor_tensor(varv[:], w[:], rse2sq[:], Op.mult)
            nc.vector.tensor_scalar_add(varv[:], varv[:], EPS)
            rstd = psm.tile([128, S, H], f32, name="rstd")
            quake(rstd[:], varv[:], [128, S, H])

            # ---- attn scaled by rstd (folded into softmax normalizer) ----
            rse2p = psm.tile([128, S, H], f32, name="rse2p")
            nc.vector.tensor_tensor(rse2p[:], rse2[:], rstd[:], Op.mult)
            attn = psm.tile([128, S, H * P], bf16, name="attn")
            nc.vector.tensor_tensor(attn.rearrange("p s (h q) -> p s h q", h=H)[:],
                                    e2[:],
                                    rse2p.unsqueeze(3).broadcast_to([128, S, H, P]),
                                    Op.mult)
            for j, st in enumerate(grp["sts"]):
                st["attn"] = attn[:, j * NSUB:(j + 1) * NSUB, :]

        def phase_b(st):
            blk, attn = st["blk"], st["attn"]
            # ---- residual x rows (bf16), only needed here ----
            xb = pxb2.tile([128, NSUB, D], bf16, name="xb")
            nc.sync.dma_start(xb[:, 0:2], xhv[blk, :, 0:2])
            nc.sync.dma_start(xb[:, 2:4], xhv[blk, :, 2:4])

            # ---- attn^T (PE transpose, free dim 128: cheap) ----
            at_ps = pp_t.tile([H * P, NSUB, 128], bf16, tag="aty", name="at_ps")
            for s in range(NSUB):
                nc.tensor.transpose(at_ps[:, s, :], attn[:, s, :], ident[:])
            attnT = psm.tile([H * P, NSUB * 128], bf16, name="attnT")
            nc.scalar.copy(attnT[:], at_ps.rearrange("p s r -> p (s r)")[:])

            # ---- y = attn @ W2 (K=16) + residual + LN2 ----
            yf = pyf.tile([128, NSUB, D], f32, name="yf")
            yo = pyo.tile([128, NSUB, D], bf16, name="yo")
            ysum = psm.tile([128, NSUB, 2], f32, name="ysum")
            yss = psm.tile([128, NSUB, 2], f32, name="yss")
            for s in range(NSUB):
                for hf in range(2):
                    y_ps = pp_y.tile([128, 512], f32, tag="ybig", name="y_ps")
                    nc.tensor.matmul(y_ps[:], attnT[:, s * 128:(s + 1) * 128],
                                     w2sb[:, hf * 512:(hf + 1) * 512],
                                     start=True, stop=True)
                    yfs = yf[:, s, hf * 512:(hf + 1) * 512]
                    nc.vector.scalar_tensor_tensor(
                        yfs, y_ps[:], 0.0, xb[:, s, hf * 512:(hf + 1) * 512],
                        Op.add, Op.add,
                        accum_out=ysum[:, s, hf:hf + 1])
                    if fb2_nz:
                        nc.gpsimd.tensor_tensor(
                            yfs, yfs, opt["fb2r"][:, hf * 512:(hf + 1) * 512], Op.add)
                    sq = psq.tile([128, 512], bf16, name="sqy", tag="sqy")
                    if s < tu.get("yss_acc", 2):
                        nc.scalar.activation(sq[:], yfs, Act.Square,
                                             accum_out=yss[:, s, hf:hf + 1])
                    else:
                        nc.vector.scalar_tensor_tensor(sq[:], yfs, 1.0, yfs,
                                                       Op.mult, Op.mult,
                                                       accum_out=yss[:, s, hf:hf + 1])

            muv = psm.tile([128, NSUB], f32, name="muv")
            nc.vector.tensor_reduce(muv[:], ysum[:], axis=mybir.AxisListType.X, op=Op.add)
            nc.vector.tensor_scalar_mul(muv[:], muv[:], 1.0 / D)
            ssv = psm.tile([128, NSUB], f32, name="ssv")
            nc.vector.tensor_reduce(ssv[:], yss[:], axis=mybir.AxisListType.X, op=Op.add)
            mu2v = psm.tile([128, NSUB], f32, name="mu2v")
            nc.vector.tensor_tensor(mu2v[:], muv[:], muv[:], Op.mult)
            var2 = psm.tile([128, NSUB], f32, name="var2")
            nc.vector.scalar_tensor_tensor(var2[:], ssv[:], 1.0 / D, mu2v[:],
                                           Op.mult, Op.subtract)
            nc.vector.tensor_scalar_add(var2[:], var2[:], EPS)
            rstd2 = psm.tile([128, NSUB], f32, name="rstd2")
            quake(rstd2[:], var2[:], [128, NSUB])

            nmr = psm.tile([128, NSUB], f32, name="nmr")
            nc.vector.scalar_tensor_tensor(nmr[:], muv[:], -1.0, rstd2[:],
                                           Op.mult, Op.mult)
            for s in range(NSUB):
                if s % 2 == 0:
                    nc.scalar.activation(yo[:, s, :], yf[:, s, :], Act.Identity,
                                         bias=nmr[:, s:s + 1],
                                         scale=rstd2[:, s:s + 1])
                else:
                    nc.vector.tensor_scalar(yo[:, s, :], yf[:, s, :],
                                            muv[:, s:s + 1], rstd2[:, s:s + 1],
                                            Op.subtract, Op.mult)
                if fln_nz:
                    nc.vector.tensor_tensor(yo[:, s, :], yo[:, s, :],
                                            opt["flngr"][:], Op.mult)
                    nc.vector.tensor_tensor(yo[:, s, :], yo[:, s, :],
                                            opt["flnbr"][:], Op.add)
            nc.sync.dma_start(yv[blk], yo[:])

        # software pipeline: chain batched over GB blocks; phase_b of group
        # g-delay runs after phase_a of group g so PE always has independent
        # q-proj matmuls queued ahead of chain-dependent final matmuls.
        delay = tu.get("delay", 1)
        GB = tu.get("gb", 2)
        ablate = tu.get("ablate", "")
        SG = GB * NSUB

        def chain_stub(grp):
            attn = psm.tile([128, SG, H * P], bf16, name="attn")
            nc.vector.memset(attn[:], 0.25)
            for j, st in enumerate(grp["sts"]):
                st["attn"] = attn[:, j * NSUB:(j + 1) * NSUB, :]

        chain_fn = chain_stub if ablate in ("nochain",) else chain
        assert nblk % GB == 0
        pending = []
        for g in range(nblk // GB):
            ssq_g = psm.tile([128, SG, H], f32, name="ssq_g")
            raw_g = psm.tile([128, SG, RW], f32, name="raw_g")
            sts = []
            for j in range(GB):
                sts.append(phase_a(g * GB + j,
                                   ssq_g[:, j * NSUB:(j + 1) * NSUB, :],
                                   raw_g[:, j * NSUB:(j + 1) * NSUB, :]))
            grp = dict(sts=sts, ssq=ssq_g, raw=raw_g, S=SG)
            pending.append(grp)
            if len(pending) > delay:
                for st in pending.pop(0)["sts"]:
                    phase_b(st)
            chain_fn(grp)
        for grp in pending:
            for st in grp["sts"]:
                phase_b(st)

    nc.compile()
    return nc


def _prepare_consts(inputs, flags):
    qb_nz, tb1_nz, tb2_nz, fln_nz, fb2_nz = flags
    EW = 32 if qb_nz else 16
    qW = np.asarray(inputs["qW"], np.float32)
    qb = np.asarray(inputs["qb"], np.float32)
    pk = np.asarray(inputs["pk"], np.float32)
    pv = np.asarray(inputs["pv"], np.float32)
    scale = np.asarray(inputs["scale"], np.float32)
    tW1 = np.asarray(inputs["tW1"], np.float32)
    tW2 = np.asarray(inputs["tW2"], np.float32)
    oW = np.asarray(inputs["oW"], np.float32)
    ob = np.asarray(inputs["ob"], np.float32)
    lng = np.asarray(inputs["lng"], np.float32)
    lnb = np.asarray(inputs["lnb"], np.float32)
    fW = np.asarray(inputs["fW"], np.float32)
    fb = np.asarray(inputs["fb"], np.float32)

    kn = pk / np.maximum(np.linalg.norm(pk, axis=-1, keepdims=True), 1e-12)
    s = np.clip(scale, 1.0, 50.0)
    knS = kn * s[:, None, None]
    qWk = np.einsum("hda,hpa->hdp", qW, knS).transpose(1, 0, 2).reshape(D, H * P)
    qW_all = qW.transpose(1, 0, 2).reshape(D, H * A)

    def _dr(w):
        # (D, C) -> (128, KD2, 2, C) DoubleRow chunk-pair layout
        C = w.shape[1]
        return np.ascontiguousarray(
            w.reshape(KD2, 2, 128, C).transpose(2, 0, 1, 3))

    ext = np.zeros((D, EW), np.float32)
    ext[:, 0:H * P] = qWk
    if qb_nz:
        qWq = np.einsum("hda,ha->hd", qW, qb).transpose(1, 0).reshape(D, H)
        ext[:, H * P:H * P + H] = qWq

    povW2 = np.einsum("hpa,hac->hpc", pv, oW) + ob[:, None, :]
    povC = povW2 - povW2.mean(axis=2, keepdims=True)         # centered (H,P,A)
    povc_bd = np.zeros((H * P, H * A), np.float32)
    for h in range(H):
        povc_bd[h * P:(h + 1) * P, h * A:(h + 1) * A] = povC[h]
    Gc = np.einsum("hpa,hqa->hpq", povC, povC) / A           # (H,P,P)
    gcb = np.broadcast_to(Gc.transpose(1, 0, 2).reshape(1, P, H, P),
                          (128, P, H, P)).astype(np.float32).copy()

    lng_flat = lng.reshape(H * A)
    fWg = fW * lng_flat[:, None]                              # (512, D)
    W2 = povc_bd @ fWg                                        # (16, D)

    tW1f = tW1[:, 0, :] / np.log(float(P))                    # (H, T)
    consts = {
        "qw8": _f8(_dr(qW_all * SC)),
        "ext8": _f8(_dr(ext * SC)),
        "w2": _bf(W2),
        "ident": _bf(np.eye(128, dtype=np.float32)),
        "gcb": gcb,
        "tw1r": np.broadcast_to(tW1f.reshape(1, H * T), (128, H * T)).astype(np.float32).copy(),
        "tw2r": np.broadcast_to(tW2[:, :, 0].reshape(1, H * T), (128, H * T)).astype(np.float32).copy(),
    }
    if qb_nz:
        qbk = np.einsum("ha,hpa->hp", qb, knS).reshape(1, H * P) * SC
        consts["qbkr"] = np.broadcast_to(qbk, (128, H * P)).astype(np.float32).copy()
        qbn2 = (qb * qb).sum(-1).reshape(1, H) * (SC * SC)
        consts["qbn2r"] = np.broadcast_to(qbn2, (128, H)).astype(np.float32).copy()
    if tb1_nz:
        tb1 = np.asarray(inputs["tb1"], np.float32).reshape(1, H * T)
        consts["tb1r"] = np.broadcast_to(tb1, (128, H * T)).astype(np.float32).copy()
    if tb2_nz:
        tb2 = np.asarray(inputs["tb2"], np.float32).reshape(1, H)
        consts["tb2r"] = np.broadcast_to(tb2, (128, H)).astype(np.float32).copy()
    if fln_nz:
        flng = np.asarray(inputs["flng"], np.float32).reshape(1, D)
        flnb = np.asarray(inputs["flnb"], np.float32).reshape(1, D)
        consts["flngr"] = np.broadcast_to(flng, (128, D)).astype(np.float32).copy()
        consts["flnbr"] = np.broadcast_to(flnb, (128, D)).astype(np.float32).copy()
    if fb2_nz:
        fb2 = (fb + lnb.reshape(H * A) @ fW).reshape(1, D)
        consts["fb2r"] = np.broadcast_to(fb2, (128, D)).astype(np.float32).copy()
    return consts


def _flags(inputs):
    lnb = np.asarray(inputs["lnb"], np.float32)
    fb = np.asarray(inputs["fb"], np.float32)
    fW = np.asarray(inputs["fW"], np.float32)
    fb2 = fb + lnb.reshape(H * A) @ fW
    return (
        bool(np.any(np.asarray(inputs["qb"]) != 0)),
        bool(np.any(np.asarray(inputs["tb1"]) != 0)),
        bool(np.any(np.asarray(inputs["tb2"]) != 0)),
        bool(np.any(np.asarray(inputs["flng"]) != 1) or np.any(np.asarray(inputs["flnb"]) != 0)),
        bool(np.any(fb2 != 0)),
    )


def make_in_maps(inputs, flags, ncores=NCORES):
    consts = _prepare_consts(inputs, flags)
    x = np.ascontiguousarray(np.asarray(inputs["x"], np.float32))
    xhi = x.astype(ml_dtypes.bfloat16)
    # xT in fp8, DoubleRow chunk-pair layout: [128, KD2, 2, B]
    x8t = x.astype(ml_dtypes.float8_e4m3).T            # (D, B)
    x8t = x8t.reshape(KD2, 2, 128, B).transpose(2, 0, 1, 3)
    in_maps = []
    for c in range(ncores):
        m = dict(consts)
        m["xhi"] = np.ascontiguousarray(xhi[c * BLOC:(c + 1) * BLOC])
        m["xt8"] = np.ascontiguousarray(x8t[:, :, :, c * BLOC:(c + 1) * BLOC])
        in_maps.append(m)
    return in_maps


def kernel(**inputs):
    from concourse.bass_utils import run_bass_kernel_spmd

    flags = _flags(inputs)
    if flags not in _cache:
        _cache[flags] = _build(flags)
    nc = _cache[flags]

    in_maps = make_in_maps(inputs, flags)
    res = run_bass_kernel_spmd(nc, in_maps, core_ids=list(range(NCORES)))
    out = np.concatenate([res.results[c]["y"] for c in range(NCORES)], axis=0)
    return out.astype(np.float32)
